# revision 1
# baseline (speedup 1.0000x reference)
"""Trainium2 Bass kernel for a 1D Kernel Neural Operator (KNO) on a regular grid.

Reference computation (N=2048 nodes, C=32 channels, DEPTH=3):
    fq = gelu([f_x, x] @ lift_W.T + lift_b)
    for i in 0..2:
        skip  = fq @ pw_W[i].T + pw_b[i]
        K_c   = sig2_c * exp(-(x_n - x_q)^2 * a_c),  a_c = 1/(2*ell2_c)
        integ = einsum('cnq,qc->nc', K, fq * w)
        fq    = skip + integ; gelu if i < 2
    out = (gelu(gelu(fq@W1.T+b1)@W2.T+b2)) @ W3.T + b3

Instead of materializing the C x N x N kernels (400M exp evaluations), we use
the factorization exp(-a(x_n-x_q)^2) = e^{-a x_n^2} e^{2 a x_n x_q} e^{-a x_q^2}
and the Taylor expansion e^{2a x_n x_q} = sum_k (2a)^k/k! x_n^k x_q^k, exact to
fp32 precision with K=64 terms (z_max = 2*a_max < 16 here; tail(z,64) < 1e-12).
Each layer's integral then becomes two small matmuls through the moment basis
V[n,k] = x_n^k:
    U      = fq ⊙ (w_q e^{-a_c x_q^2})              [N,C]
    M[k,c] = sum_q V[q,k] U[q,c]                    [K,C]   (PE matmul)
    Mt     = M ⊙ B,  B[k,c] = (2a_c)^k/k! = exp(-2k*log_ell - ln k!)
    integ  = (s2_c e^{-a_c x_n^2}) ⊙ (V @ Mt)       [N,C]   (PE matmul)

Data layout: the [N,C] state lives channel-transposed in a 4-chunk stack
fqT4[c + 32j, n'] = fq[512j + n', c], a single [128, 512] SBUF tile. This makes
biases per-partition scalars, gelu a single ACT op, and channel mixing 4 PE
matmuls. The moment contraction needs q on partitions, so each layer does 4
full 128x128 PE transposes of fqT4 back to natural layout.

Sharding: the whole problem is ~30-50us of dependent small ops, so all 8 cores
run identical replicas (collectives would cost more than they save); the output
is taken from core 0.
"""

import numpy as np

import concourse.bass as bass
import concourse.tile as tile
from concourse import bacc, mybir
from concourse.bass_utils import run_bass_kernel_spmd

N = 2048
C = 32
K = 64
DEPTH = 3
NCORES = 8
NT = N // 128            # 16 n-tiles of 128
NCHUNK = N // 512        # 4 chunks of 512
F32 = mybir.dt.float32
AF = mybir.ActivationFunctionType
ALU = mybir.AluOpType

_CACHE = {}


def _declare_inputs(nc):
    d = {}
    d["lift_inT"] = nc.dram_tensor("lift_inT", [3, N], F32, kind="ExternalInput")
    d["lift_WTb"] = nc.dram_tensor("lift_WTb", [3, C], F32, kind="ExternalInput")
    d["x_row"] = nc.dram_tensor("x_row", [1, N], F32, kind="ExternalInput")
    d["x_tp"] = nc.dram_tensor("x_tp", [NT, 128], F32, kind="ExternalInput")
    d["w_row"] = nc.dram_tensor("w_row", [1, N], F32, kind="ExternalInput")
    d["kle"] = nc.dram_tensor("kle", [1, DEPTH * C], F32, kind="ExternalInput")
    d["kls4"] = nc.dram_tensor("kls4", [DEPTH, 128], F32, kind="ExternalInput")
    d["pw_WT4"] = nc.dram_tensor("pw_WT4", [DEPTH, 128, C], F32, kind="ExternalInput")
    d["pw_b4"] = nc.dram_tensor("pw_b4", [DEPTH, 128], F32, kind="ExternalInput")
    d["p1_WT4"] = nc.dram_tensor("p1_WT4", [128, C], F32, kind="ExternalInput")
    d["p1_b4"] = nc.dram_tensor("p1_b4", [1, 128], F32, kind="ExternalInput")
    d["p2_WT4"] = nc.dram_tensor("p2_WT4", [128, C], F32, kind="ExternalInput")
    d["p2_b4"] = nc.dram_tensor("p2_b4", [1, 128], F32, kind="ExternalInput")
    d["p3_WT4"] = nc.dram_tensor("p3_WT4", [128, 1], F32, kind="ExternalInput")
    d["p3_b"] = nc.dram_tensor("p3_b", [1, 1], F32, kind="ExternalInput")
    d["identity"] = nc.dram_tensor("identity", [128, 128], F32, kind="ExternalInput")
    d["krow"] = nc.dram_tensor("krow", [128, K], F32, kind="ExternalInput")
    d["kcol_m2"] = nc.dram_tensor("kcol_m2", [K, 1], F32, kind="ExternalInput")
    d["lnfact"] = nc.dram_tensor("lnfact", [K, 1], F32, kind="ExternalInput")
    d["ones96"] = nc.dram_tensor("ones96", [1, DEPTH * C], F32, kind="ExternalInput")
    return d


def build_program(nc):
    din = _declare_inputs(nc)
    out_dram = nc.dram_tensor("out", [1, N], F32, kind="ExternalOutput")

    with tile.TileContext(nc) as tc:
        with (
            tc.tile_pool(name="const", bufs=1) as cp,
            tc.tile_pool(name="work", bufs=2) as wp,
            tc.tile_pool(name="psum", bufs=1, space="PSUM") as pp,
            tc.tile_pool(name="psum_m", bufs=2, space="PSUM") as pmp,
        ):
            # ---------------- loads ----------------
            lift_inT = cp.tile([3, N], F32, tag="lift_inT")
            nc.sync.dma_start(lift_inT[:], din["lift_inT"][:])
            lift_WTb = cp.tile([3, C], F32, tag="lift_WTb")
            nc.sync.dma_start(lift_WTb[:], din["lift_WTb"][:])
            augN = cp.tile([2, N], F32, tag="augN")  # row0 -> x^2, row1 -> ln w
            nc.sync.dma_start(augN[0:1, :], din["x_row"][:])
            wln = cp.tile([1, N], F32, tag="wln")
            nc.sync.dma_start(wln[:], din["w_row"][:])
            xcols = cp.tile([128, NT], F32, tag="xcols")
            nc.sync.dma_start(xcols[:], din["x_tp"][:].rearrange("t p -> p t"))
            Lrow = cp.tile([1, DEPTH * C], F32, tag="Lrow")
            nc.sync.dma_start(Lrow[:], din["kle"][:])
            sigcol = [cp.tile([128, 1], F32, name=f"sigcol{i}", tag=f"sigcol{i}") for i in range(DEPTH)]
            for i in range(DEPTH):
                nc.sync.dma_start(sigcol[i][:], din["kls4"][i : i + 1, :].rearrange("a b -> b a"))
            pw_WT4 = [cp.tile([128, C], F32, name=f"pwWT{i}", tag=f"pwWT{i}") for i in range(DEPTH)]
            for i in range(DEPTH):
                nc.sync.dma_start(pw_WT4[i][:], din["pw_WT4"][i, :, :])
            pwbcol = [cp.tile([128, 1], F32, name=f"pwb{i}", tag=f"pwb{i}") for i in range(DEPTH)]
            for i in range(DEPTH):
                nc.sync.dma_start(pwbcol[i][:], din["pw_b4"][i : i + 1, :].rearrange("a b -> b a"))
            p1_WT4 = cp.tile([128, C], F32, tag="p1w")
            nc.sync.dma_start(p1_WT4[:], din["p1_WT4"][:])
            p1bcol = cp.tile([128, 1], F32, tag="p1b")
            nc.sync.dma_start(p1bcol[:], din["p1_b4"][:].rearrange("a b -> b a"))
            p2_WT4 = cp.tile([128, C], F32, tag="p2w")
            nc.sync.dma_start(p2_WT4[:], din["p2_WT4"][:])
            p2bcol = cp.tile([128, 1], F32, tag="p2b")
            nc.sync.dma_start(p2bcol[:], din["p2_b4"][:].rearrange("a b -> b a"))
            p3_WT4 = cp.tile([128, 1], F32, tag="p3w")
            nc.sync.dma_start(p3_WT4[:], din["p3_WT4"][:])
            p3b = cp.tile([1, 1], F32, tag="p3b")
            nc.sync.dma_start(p3b[:], din["p3_b"][:])
            ident = cp.tile([128, 128], F32, tag="ident")
            nc.sync.dma_start(ident[:], din["identity"][:])
            krow = cp.tile([128, K], F32, tag="krow")
            nc.sync.dma_start(krow[:], din["krow"][:])
            kcol_m2 = cp.tile([K, 1], F32, tag="kcol_m2")
            nc.sync.dma_start(kcol_m2[:], din["kcol_m2"][:])
            lnfact = cp.tile([K, 1], F32, tag="lnfact")
            nc.sync.dma_start(lnfact[:], din["lnfact"][:])

            # ---------------- phase 0: exp/log precompute ----------------
            # augN row0: x -> x^2 (in place); row1: w -> ln w (in place)
            nc.vector.tensor_mul(augN[0:1, :], augN[0:1, :], augN[0:1, :])
            nc.scalar.activation(wln[:], wln[:], AF.Ln)
            nc.sync.dma_start(augN[1:2, :], wln[:])

            # ln x (clamped away from 0)
            nc.vector.tensor_scalar_max(xcols[:], xcols[:], 1e-35)
            lnx = cp.tile([128, NT], F32, tag="lnx")
            nc.scalar.activation(lnx[:], xcols[:], AF.Ln)

            # neg_a[0, 32*i + c] = -a_c(layer i) = -0.5 * exp(-2 * log_ell)
            neg_a = cp.tile([1, DEPTH * C], F32, tag="neg_a")
            nc.scalar.activation(neg_a[:], Lrow[:], AF.Exp, scale=-2.0)
            nc.vector.tensor_scalar_mul(neg_a[:], neg_a[:], -0.5)
            # rhs_all rows: (neg_a ; ones)
            rhs_all = cp.tile([2, DEPTH * C], F32, tag="rhs_all")
            nc.sync.dma_start(rhs_all[0:1, :], neg_a[0:1, :])
            nc.sync.dma_start(rhs_all[1:2, :], din["ones96"][:])

            # V[p, 64t + k] = x_{128t+p}^k = exp(k * ln x)
            vpre = wp.tile([128, NT * K], F32, tag="vpre")
            for t in range(NT):
                nc.vector.tensor_scalar_mul(
                    vpre[:, K * t : K * (t + 1)], krow[:], lnx[:, t : t + 1]
                )
            V = cp.tile([128, NT * K], F32, tag="V")
            nc.scalar.activation(V[:], vpre[:], AF.Exp)

            # VT[k, n] = x_n^k via 8 PE transposes of 128-col chunks of V
            VT = cp.tile([K, N], F32, tag="VT")
            for m in range(NT * K // 128):  # 8 chunks, each covering tiles 2m, 2m+1
                trp = pp.tile([128, 128], F32, tag="trp", padded_shape=[128, 512])
                nc.tensor.transpose(trp[:], V[:, 128 * m : 128 * (m + 1)], ident[:])
                for s in range(2):
                    t = 2 * m + s
                    nc.vector.tensor_copy(
                        VT[:, 128 * t : 128 * (t + 1)], trp[64 * s : 64 * (s + 1), :]
                    )

            # tmpN_i[p, 128m + 32j + c] = w_q * exp(-a_c x_q^2), q = 512j + 128m + p
            # (column order matches the per-layer transpose output, see layer loop)
            tmpN = []
            for i in range(DEPTH):
                expo = pp.tile([128, 512], F32, tag="expo")
                for t in range(NT):
                    m, j = t % 4, t // 4
                    nc.tensor.matmul(
                        expo[:, 128 * m + 32 * j : 128 * m + 32 * j + 32],
                        augN[:, 128 * t : 128 * (t + 1)],
                        rhs_all[:, C * i : C * (i + 1)],
                        start=True,
                        stop=True,
                    )
                ti = cp.tile([128, 512], F32, name=f"tmpN{i}", tag=f"tmpN{i}")
                nc.scalar.activation(ti[:], expo[:], AF.Exp)
                tmpN.append(ti)

            # Es_i[c + 32j, n'] = sig2_c * exp(-a_c x_n^2), n = 512j + n'
            Es = []
            for i in range(DEPTH):
                esp = pp.tile([128, 512], F32, tag="expo")
                for j in range(NCHUNK):
                    nc.tensor.matmul(
                        esp[32 * j : 32 * (j + 1), :],
                        neg_a[0:1, C * i : C * (i + 1)],
                        augN[0:1, 512 * j : 512 * (j + 1)],
                        start=True,
                        stop=True,
                        tile_position=(0, 32 * j),
                    )
                ei = cp.tile([128, 512], F32, name=f"Es{i}", tag=f"Es{i}")
                nc.scalar.activation(ei[:], esp[:], AF.Exp, bias=sigcol[i][:, 0:1])
                Es.append(ei)

            # B_i[k, c] = exp(-2k * log_ell[i,c] - ln k!) = (2 a_c)^k / k!
            ones64 = cp.tile([1, K], F32, tag="ones64")
            nc.vector.memset(ones64[:], 1.0)
            B = []
            for i in range(DEPTH):
                lb = pmp.tile([K, C], F32, tag="Mp", bufs=1)
                nc.tensor.matmul(
                    lb[:], ones64[:], Lrow[0:1, C * i : C * (i + 1)], start=True, stop=True
                )
                bpre = wp.tile([K, C], F32, tag="bpre")
                nc.vector.tensor_scalar(
                    bpre[:], lb[:], kcol_m2[:, 0:1], lnfact[:, 0:1], ALU.mult, ALU.subtract
                )
                bi = cp.tile([K, C], F32, name=f"B{i}", tag=f"B{i}")
                nc.scalar.activation(bi[:], bpre[:], AF.Exp)
                B.append(bi)

            # ---------------- lift ----------------
            liftp = pp.tile([128, 512], F32, tag="mixp")
            for j in range(NCHUNK):
                nc.tensor.matmul(
                    liftp[32 * j : 32 * (j + 1), :],
                    lift_WTb[:],
                    lift_inT[:, 512 * j : 512 * (j + 1)],
                    start=True,
                    stop=True,
                    tile_position=(0, 32 * j),
                )
            fq = wp.tile([128, 512], F32, tag="fq")
            nc.scalar.activation(fq[:], liftp[:], AF.Gelu_apprx_tanh)

            # ---------------- KNO layers ----------------
            for i in range(DEPTH):
                # skip^T (no bias; bias added in the combine)
                skp = pp.tile([128, 512], F32, tag="mixp")
                for j in range(NCHUNK):
                    nc.tensor.matmul(
                        skp[32 * j : 32 * (j + 1), :],
                        pw_WT4[i][32 * j : 32 * (j + 1), :],
                        fq[32 * j : 32 * (j + 1), :],
                        start=True,
                        stop=True,
                        tile_position=(32 * j, 32 * j),
                    )
                # transpose back to natural layout and form U = fq_nat * tmpN
                trp = pp.tile([128, 512], F32, tag="trp")
                for m in range(4):
                    nc.tensor.transpose(
                        trp[:, 128 * m : 128 * (m + 1)],
                        fq[:, 128 * m : 128 * (m + 1)],
                        ident[:],
                    )
                U = wp.tile([128, 512], F32, tag="U")
                for m in range(4):
                    nc.vector.tensor_mul(
                        U[:, 128 * m : 128 * (m + 1)],
                        trp[:, 128 * m : 128 * (m + 1)],
                        tmpN[i][:, 128 * m : 128 * (m + 1)],
                    )
                # moments M[k,c] = sum_q V[q,k] U[q,c]
                Mp = pmp.tile([K, C], F32, tag="Mp", bufs=1)
                for t in range(NT):
                    m, j = t % 4, t // 4
                    nc.tensor.matmul(
                        Mp[:],
                        V[:, K * t : K * (t + 1)],
                        U[:, 128 * m + 32 * j : 128 * m + 32 * j + 32],
                        start=(t == 0),
                        stop=(t == NT - 1),
                    )
                Mt = wp.tile([K, C], F32, tag="Mt")
                nc.vector.tensor_mul(Mt[:], Mp[:], B[i][:])
                # eval: PT[c + 32j, n'] = sum_k Mt[k,c] VT[k, 512j + n']
                PT = pp.tile([128, 512], F32, tag="PT")
                for j in range(NCHUNK):
                    nc.tensor.matmul(
                        PT[32 * j : 32 * (j + 1), :],
                        Mt[:],
                        VT[:, 512 * j : 512 * (j + 1)],
                        start=True,
                        stop=True,
                        tile_position=(0, 32 * j),
                    )
                # combine: fq_next = gelu(skip + pw_b + Es * PT)
                z = wp.tile([128, 512], F32, tag="z")
                nc.vector.tensor_mul(z[:], PT[:], Es[i][:])
                pre = wp.tile([128, 512], F32, tag="fq")
                nc.vector.scalar_tensor_tensor(
                    pre[:], z[:], pwbcol[i][:, 0:1], skp[:], ALU.add, ALU.add
                )
                if i < DEPTH - 1:
                    fq2 = wp.tile([128, 512], F32, tag="fq")
                    nc.scalar.activation(fq2[:], pre[:], AF.Gelu_apprx_tanh)
                    fq = fq2
                else:
                    fq = pre

            # ---------------- projection head ----------------
            for W4, bcol in ((p1_WT4, p1bcol), (p2_WT4, p2bcol)):
                ppj = pp.tile([128, 512], F32, tag="mixp")
                for j in range(NCHUNK):
                    nc.tensor.matmul(
                        ppj[32 * j : 32 * (j + 1), :],
                        W4[32 * j : 32 * (j + 1), :],
                        fq[32 * j : 32 * (j + 1), :],
                        start=True,
                        stop=True,
                        tile_position=(32 * j, 32 * j),
                    )
                nxt = wp.tile([128, 512], F32, tag="fq")
                nc.scalar.activation(nxt[:], ppj[:], AF.Gelu_apprx_tanh, bias=bcol[:, 0:1])
                fq = nxt

            outsb = wp.tile([1, N], F32, tag="outsb")
            for j in range(NCHUNK):
                p3p = pmp.tile([1, 512], F32, tag="p3p")
                nc.tensor.matmul(
                    p3p[:],
                    p3_WT4[32 * j : 32 * (j + 1), 0:1],
                    fq[32 * j : 32 * (j + 1), :],
                    start=True,
                    stop=True,
                    tile_position=(32 * j, 0),
                )
                nc.vector.tensor_scalar_add(
                    outsb[0:1, 512 * j : 512 * (j + 1)], p3p[:], p3b[0:1, 0:1]
                )
            nc.sync.dma_start(out_dram[:], outsb[:])

    return nc


def get_nc():
    if "nc" not in _CACHE:
        nc = bacc.Bacc("TRN2", target_bir_lowering=False, debug=False, num_devices=NCORES)
        build_program(nc)
        nc.compile()
        _CACHE["nc"] = nc
    return _CACHE["nc"]


def make_in_map(
    f_x, x_grid, q_weights, lift_W, lift_b, pw_W, pw_b, ker_log_ell, ker_log_sigma,
    proj1_W, proj1_b, proj2_W, proj2_b, proj3_W, proj3_b,
):
    f4 = lambda a: np.ascontiguousarray(np.asarray(a, dtype=np.float32))
    f_x, x_grid, q_weights = f4(f_x), f4(x_grid), f4(q_weights)
    x = x_grid.reshape(N)
    ks = np.arange(K, dtype=np.float64)
    lnfact = np.concatenate([[0.0], np.cumsum(np.log(np.arange(1, K)))])
    return {
        "lift_inT": f4(np.stack([f_x.reshape(N), x, np.ones(N, np.float32)])),
        "lift_WTb": f4(np.vstack([f4(lift_W).T, f4(lift_b)[None, :]])),
        "x_row": f4(x.reshape(1, N)),
        "x_tp": f4(x.reshape(NT, 128)),
        "w_row": f4(q_weights.reshape(1, N)),
        "kle": f4(ker_log_ell).reshape(1, DEPTH * C),
        "kls4": f4(np.tile(2.0 * f4(ker_log_sigma), (1, 4))),
        "pw_WT4": f4(np.tile(f4(pw_W).transpose(0, 2, 1), (1, 4, 1))),
        "pw_b4": f4(np.tile(f4(pw_b), (1, 4))),
        "p1_WT4": f4(np.tile(f4(proj1_W).T, (4, 1))),
        "p1_b4": f4(np.tile(f4(proj1_b), 4).reshape(1, 128)),
        "p2_WT4": f4(np.tile(f4(proj2_W).T, (4, 1))),
        "p2_b4": f4(np.tile(f4(proj2_b), 4).reshape(1, 128)),
        "p3_WT4": f4(np.tile(f4(proj3_W).T, (4, 1))),
        "p3_b": f4(proj3_b).reshape(1, 1),
        "identity": np.eye(128, dtype=np.float32),
        "ones96": np.ones((1, DEPTH * C), dtype=np.float32),
        "krow": np.broadcast_to(ks.astype(np.float32), (128, K)).copy(),
        "kcol_m2": (-2.0 * ks).astype(np.float32).reshape(K, 1),
        "lnfact": lnfact.astype(np.float32).reshape(K, 1),
    }


def kernel(**inputs) -> np.ndarray:
    nc = get_nc()
    in_map = make_in_map(**inputs)
    res = run_bass_kernel_spmd(nc, [in_map] * NCORES, list(range(NCORES)))
    return np.asarray(res.results[0]["out"], dtype=np.float32).reshape(N)



# revision 11
# speedup vs baseline: 2.5298x; 2.5298x over previous
"""Trainium2 Bass kernel for a 1D Kernel Neural Operator (KNO) on a regular grid.

Reference computation (N=2048 nodes, C=32 channels, DEPTH=3):
    fq = gelu([f_x, x] @ lift_W.T + lift_b)
    for i in 0..2:
        skip  = fq @ pw_W[i].T + pw_b[i]
        K_c   = sig2_c * exp(-(x_n - x_q)^2 * a_c),  a_c = 1/(2*ell2_c)
        integ = einsum('cnq,qc->nc', K, fq * w)
        fq    = skip + integ; gelu if i < 2
    out = (gelu(gelu(fq@W1.T+b1)@W2.T+b2)) @ W3.T + b3

Instead of materializing the C x N x N kernels, we use the factorization
exp(-a(x_n-x_q)^2) = e^{-a x_n^2} e^{2 a x_n x_q} e^{-a x_q^2} and the Taylor
expansion e^{2a x_n x_q} = sum_k (2a)^k/k! x_n^k x_q^k (K=32 terms; the
neglected tail is < ~1e-5 relative for the a-range this problem generates).
Each layer's integral becomes two small matmuls through the moment basis:
    U      = fq ⊙ e^{-a_c x_q^2}                     [N,C]
    M[k,c] = sum_q (w_q x_q^k) U[q,c]               [K,C]   (PE matmul)
    Mt     = M ⊙ B,  B[k,c] = (2a_c)^k/k!
    integ  = (s2_c e^{-a_c x_n^2}) ⊙ (V @ Mt)       [N,C]   (PE matmul)

All x-grid/parameter-derived constants (moment basis V, its transpose, the
exponential prefactor tensors, scaled weights) are precomputed host-side in
float64 and shipped in three batched DMA blocks, so the device program is just
the f_x-dependent dataflow: ~90 engine instructions, all matmuls in bf16
(single-pass PE mode, fp32 PSUM accumulate), gelu the only activation function.

Data layout: the [N,C] state lives channel-transposed in a 4-chunk stack
fqT4[c + 32j, n'] = fq[512j + n', c], a single [128, 512] SBUF tile. Channel
mixing / eval / final projection each run as 4 concurrent quadrant matmuls.

Sharding: the whole problem is a ~30us chain of dependent small ops, so all 8
cores run identical replicas (collectives would cost more than they save); the
output is taken from core 0.
"""

import numpy as np

import concourse.bass as bass
import concourse.tile as tile
from concourse import bacc, mybir
from concourse.bass_utils import run_bass_kernel_spmd

N = 2048
C = 32
K = 32
DEPTH = 3
NCORES = 8
NT = N // 128             # 16 q-tiles of 128
NCHUNK = N // 512         # 4 chunks of 512
F32 = mybir.dt.float32
BF16 = mybir.dt.bfloat16
NPBF16 = mybir.dt.np(mybir.dt.bfloat16)
AF = mybir.ActivationFunctionType
ALU = mybir.AluOpType

# pack1 column offsets
P_IDENT = 0
P_V4 = P_IDENT + 128          # V4[p, 32t+k] = w_q x_q^k, q = 128t+p
P_VT4 = P_V4 + NT * K         # VT4[32s+k, n'] = x_{512s+n'}^k
P_PWW = P_VT4 + 512           # pwWT3[c+32j, 32i+c'] = pw_W[i][c',c]
P_P1W = P_PWW + 3 * C
P_P2W = P_P1W + C
P_B4 = P_P2W + C              # B4_3[32s+k, 32i+c] = (2a_ic)^k/k!
P_P3W = P_B4 + 3 * C
P_PWB = P_P3W + 1             # pwb3[c+32j, i] = pw_b[i][c]
P_P1B = P_PWB + 3
P_P2B = P_P1B + 1
P_P3B = P_P2B + 1
P_LB = P_P3B + 1              # liftb4[c+32j] = lift_b[c]
P_LW = P_LB + 1               # liftW[r, c] = lift_W[c, r], rows 0..1
W1 = P_LW + C
WC = 8                        # packc (f32): pwb3 | p1b | p2b | p3b | liftb
PC_PWB = 0
PC_P1B = 3
PC_P2B = 4
PC_P3B = 5
PC_LB = 6
W2 = 1024                     # pack2: tmpN_0 | Es_0
W3 = 4 * 512                  # pack3: tmpN_1 | Es_1 | tmpN_2 | Es_2

_CACHE = {}


def build_program(nc):
    pack1 = nc.dram_tensor("pack1", [128, W1], BF16, kind="ExternalInput")
    pack2 = nc.dram_tensor("pack2", [128, W2], BF16, kind="ExternalInput")
    pack3 = nc.dram_tensor("pack3", [128, W3], BF16, kind="ExternalInput")
    lift_in = nc.dram_tensor("lift_in", [2, N], BF16, kind="ExternalInput")
    packc = nc.dram_tensor("packc", [128, WC], F32, kind="ExternalInput")
    out_dram = nc.dram_tensor("out", [1, N], F32, kind="ExternalOutput")

    with tile.TileContext(nc) as tc:
        with (
            tc.tile_pool(name="const", bufs=1) as cp,
            tc.tile_pool(name="work", bufs=2) as wp,
            tc.tile_pool(name="psA", bufs=2, space="PSUM") as ppa,
            tc.tile_pool(name="psB", bufs=2, space="PSUM") as ppb,
            tc.tile_pool(name="psC", bufs=2, space="PSUM") as ppc,
            tc.tile_pool(name="psS", bufs=2, space="PSUM") as pps,
        ):
            # ------- batched input loads on four different queues -------
            p1 = cp.tile([128, W1], BF16, tag="p1")
            nc.sync.dma_start(p1[:], pack1[:])
            p2 = cp.tile([128, W2], BF16, tag="p2")
            nc.gpsimd.dma_start(p2[:], pack2[:])
            pc = cp.tile([128, WC], F32, tag="pc")
            nc.gpsimd.dma_start(pc[:], packc[:])
            liftsb = cp.tile([2, N], BF16, tag="liftsb")
            nc.scalar.dma_start(liftsb[:], lift_in[:])
            p3 = cp.tile([128, W3], BF16, tag="p3")
            nc.sync.dma_start(p3[:], pack3[:])

            ident = p1[:, P_IDENT : P_IDENT + 128]
            V4 = p1[:, P_V4 : P_V4 + NT * K]
            VT4 = p1[:, P_VT4 : P_VT4 + 512]

            def tmpN(i):
                src = p2 if i == 0 else p3
                off = 0 if i == 0 else 1024 * (i - 1)
                return src[:, off : off + 512]

            def Es(i):
                src = p2 if i == 0 else p3
                off = 512 if i == 0 else 1024 * (i - 1) + 512
                return src[:, off : off + 512]

            # ---------------- lift ----------------
            liftp = ppa.tile([128, 512], F32, tag="mix")
            for j in range(NCHUNK):
                nc.tensor.matmul(
                    liftp[32 * j : 32 * (j + 1), :],
                    p1[0:2, P_LW : P_LW + C],
                    liftsb[:, 512 * j : 512 * (j + 1)],
                    start=True,
                    stop=True,
                    tile_position=(0, 32 * j),
                )
            fq = wp.tile([128, 512], BF16, tag="fq")
            nc.scalar.activation(
                fq[:], liftp[:], AF.Gelu_apprx_tanh, bias=pc[:, PC_LB : PC_LB + 1]
            )

            # ---------------- KNO layers ----------------
            for i in range(DEPTH):
                # skip^T (bias folded into the combine/gelu)
                skp = ppa.tile([128, 512], F32, tag="mix")
                for j in range(NCHUNK):
                    nc.tensor.matmul(
                        skp[32 * j : 32 * (j + 1), :],
                        p1[32 * j : 32 * (j + 1), P_PWW + C * i : P_PWW + C * (i + 1)],
                        fq[32 * j : 32 * (j + 1), :],
                        start=True,
                        stop=True,
                        tile_position=(32 * j, 32 * j),
                    )
                # transpose back to natural layout, U = fq_nat * e^{-a x_q^2}
                trp = ppb.tile([128, 512], BF16, tag="trp")
                for m in range(4):
                    nc.tensor.transpose(
                        trp[:, 128 * m : 128 * (m + 1)],
                        fq[:, 128 * m : 128 * (m + 1)],
                        ident,
                    )
                U = wp.tile([128, 512], BF16, tag="U")
                nc.vector.tensor_mul(U[:], trp[:], tmpN(i))
                # moments M[k,c] = sum_q (w_q x_q^k) U[q,c]
                Mp = pps.tile([K, C], F32, tag="Mp")
                for t in range(NT):
                    m, j = t % 4, t // 4
                    nc.tensor.matmul(
                        Mp[:],
                        V4[:, K * t : K * (t + 1)],
                        U[:, 128 * m + 32 * j : 128 * m + 32 * j + 32],
                        start=(t == 0),
                        stop=(t == NT - 1),
                    )
                # Mt4: B-scaled moments replicated into all 4 partition blocks
                Mt4 = wp.tile([128, C], BF16, tag="Mt4")
                for s in range(4):
                    nc.vector.tensor_mul(
                        Mt4[32 * s : 32 * (s + 1), :],
                        Mp[:],
                        p1[32 * s : 32 * (s + 1), P_B4 + C * i : P_B4 + C * (i + 1)],
                    )
                # eval: PT[c + 32s, n'] = sum_k Mt[k,c] x_n^k, n = 512s + n'
                PT = ppc.tile([128, 512], F32, tag="PT")
                for s in range(4):
                    nc.tensor.matmul(
                        PT[32 * s : 32 * (s + 1), :],
                        Mt4[32 * s : 32 * (s + 1), :],
                        VT4[32 * s : 32 * (s + 1), :],
                        start=True,
                        stop=True,
                        tile_position=(32 * s, 32 * s),
                    )
                # combine: fq_next = gelu(skip + pw_b + Es * PT)
                z = wp.tile([128, 512], F32, tag="z")
                nc.vector.tensor_mul(z[:], PT[:], Es(i))
                if i < DEPTH - 1:
                    pre = wp.tile([128, 512], F32, tag="pre")
                    nc.vector.tensor_add(pre[:], z[:], skp[:])
                    fq = wp.tile([128, 512], BF16, tag="fq")
                    nc.scalar.activation(
                        fq[:], pre[:], AF.Gelu_apprx_tanh,
                        bias=pc[:, PC_PWB + i : PC_PWB + i + 1],
                    )
                else:
                    fq = wp.tile([128, 512], BF16, tag="fq")
                    nc.vector.scalar_tensor_tensor(
                        fq[:], z[:], pc[:, PC_PWB + i : PC_PWB + i + 1], skp[:],
                        ALU.add, ALU.add,
                    )

            # ---------------- projection head ----------------
            for woff, boff in ((P_P1W, PC_P1B), (P_P2W, PC_P2B)):
                ppj = ppa.tile([128, 512], F32, tag="mix")
                for j in range(NCHUNK):
                    nc.tensor.matmul(
                        ppj[32 * j : 32 * (j + 1), :],
                        p1[32 * j : 32 * (j + 1), woff : woff + C],
                        fq[32 * j : 32 * (j + 1), :],
                        start=True,
                        stop=True,
                        tile_position=(32 * j, 32 * j),
                    )
                nxt = wp.tile([128, 512], BF16, tag="fq")
                nc.scalar.activation(
                    nxt[:], ppj[:], AF.Gelu_apprx_tanh, bias=pc[:, boff : boff + 1]
                )
                fq = nxt

            p3p = ppc.tile([128, 512], F32, tag="PT")
            for j in range(NCHUNK):
                nc.tensor.matmul(
                    p3p[32 * j : 32 * j + 1, :],
                    p1[32 * j : 32 * (j + 1), P_P3W : P_P3W + 1],
                    fq[32 * j : 32 * (j + 1), :],
                    start=True,
                    stop=True,
                    tile_position=(32 * j, 32 * j),
                )
            outsb = wp.tile([1, N], F32, tag="outsb")
            for j in range(NCHUNK):
                nc.vector.tensor_scalar_add(
                    outsb[0:1, 512 * j : 512 * (j + 1)],
                    p3p[32 * j : 32 * j + 1, :],
                    pc[32 * j : 32 * j + 1, PC_P3B : PC_P3B + 1],
                )
            nc.sync.dma_start(out_dram[:], outsb[:])

    return nc


def get_nc():
    if "nc" not in _CACHE:
        nc = bacc.Bacc("TRN2", target_bir_lowering=False, debug=False, num_devices=NCORES)
        build_program(nc)
        nc.compile()
        _CACHE["nc"] = nc
    return _CACHE["nc"]


def make_in_map(
    f_x, x_grid, q_weights, lift_W, lift_b, pw_W, pw_b, ker_log_ell, ker_log_sigma,
    proj1_W, proj1_b, proj2_W, proj2_b, proj3_W, proj3_b,
):
    f4 = lambda a: np.asarray(a, dtype=np.float64)
    x = f4(x_grid).reshape(N)
    w = f4(q_weights).reshape(N)
    ks = np.arange(K, dtype=np.float64)
    lnfact = np.concatenate([[0.0], np.cumsum(np.log(np.arange(1.0, K)))])
    a = 0.5 * np.exp(-2.0 * f4(ker_log_ell))        # [DEPTH, C]
    sig2 = np.exp(2.0 * f4(ker_log_sigma))          # [DEPTH, C]

    pack1 = np.zeros((128, W1), dtype=np.float32)
    pack1[:, P_IDENT : P_IDENT + 128] = np.eye(128, dtype=np.float32)
    # V4[p, 32t+k] = w_q x_q^k, q = 128t + p
    V = w[:, None] * np.power(x[:, None], ks[None, :])          # [N, K]
    pack1[:, P_V4 : P_V4 + NT * K] = (
        V.reshape(NT, 128, K).transpose(1, 0, 2).reshape(128, NT * K)
    ).astype(np.float32)
    # VT4[32s+k, n'] = x_{512s+n'}^k
    VT = np.power(x[None, :], ks[:, None])                      # [K, N]
    pack1[:, P_VT4 : P_VT4 + 512] = (
        VT.reshape(K, NCHUNK, 512).transpose(1, 0, 2).reshape(128, 512)
    ).astype(np.float32)
    # pwWT3[c+32j, 32i+c'] = pw_W[i][c', c]
    pwT = f4(pw_W).transpose(0, 2, 1)                           # [D, C, C']
    pack1[:, P_PWW : P_PWW + 3 * C] = np.tile(
        pwT.transpose(1, 0, 2).reshape(C, 3 * C), (4, 1)
    ).astype(np.float32)
    pack1[:, P_P1W : P_P1W + C] = np.tile(f4(proj1_W).T, (4, 1)).astype(np.float32)
    pack1[:, P_P2W : P_P2W + C] = np.tile(f4(proj2_W).T, (4, 1)).astype(np.float32)
    # B4_3[32s+k, 32i+c] = (2 a_ic)^k / k!
    B = np.exp(ks[None, :, None] * np.log(2.0 * a)[:, None, :] - lnfact[None, :, None])
    pack1[:, P_B4 : P_B4 + 3 * C] = np.tile(
        B.transpose(1, 0, 2).reshape(K, 3 * C), (4, 1)
    ).astype(np.float32)
    pack1[:, P_P3W : P_P3W + 1] = np.tile(f4(proj3_W).T, (4, 1)).astype(np.float32)
    pack1[0:2, P_LW : P_LW + C] = f4(lift_W).T.astype(np.float32)
    packc = np.zeros((128, WC), dtype=np.float32)
    packc[:, PC_PWB : PC_PWB + 3] = np.tile(f4(pw_b).T, (4, 1)).astype(np.float32)
    packc[:, PC_P1B] = np.tile(f4(proj1_b), 4).astype(np.float32)
    packc[:, PC_P2B] = np.tile(f4(proj2_b), 4).astype(np.float32)
    packc[:, PC_P3B] = np.float32(f4(proj3_b)[0])
    packc[:, PC_LB] = np.tile(f4(lift_b), 4).astype(np.float32)

    # exponential prefactors per layer
    # tmpN[p, 128m+32j+c] = exp(-a_c x_q^2), q = 512j+128m+p
    # Es[c+32j, n'] = sig2_c exp(-a_c x_n^2), n = 512j+n'
    x2 = x * x
    packs = []
    for i in range(DEPTH):
        eN = np.exp(-x2[:, None] * a[i][None, :])               # [N, C]
        tm = (
            eN.reshape(NCHUNK, 4, 128, C).transpose(2, 1, 0, 3).reshape(128, 512)
        ).astype(np.float32)
        es = (
            (sig2[i][None, :] * eN).reshape(NCHUNK, 512, C)
            .transpose(0, 2, 1).reshape(128, 512)
        ).astype(np.float32)
        packs.append((tm, es))
    pack2 = np.concatenate(packs[0], axis=1)
    pack3 = np.concatenate([t for pr in packs[1:] for t in pr], axis=1)

    lift_arr = np.stack(
        [np.asarray(f_x, np.float32).reshape(N), x.astype(np.float32)]
    )
    return {
        "pack1": pack1.astype(NPBF16),
        "pack2": np.ascontiguousarray(pack2).astype(NPBF16),
        "pack3": np.ascontiguousarray(pack3).astype(NPBF16),
        "lift_in": np.ascontiguousarray(lift_arr).astype(NPBF16),
        "packc": packc,
    }


def kernel(**inputs) -> np.ndarray:
    nc = get_nc()
    in_map = make_in_map(**inputs)
    res = run_bass_kernel_spmd(nc, [in_map] * NCORES, list(range(NCORES)))
    return np.asarray(res.results[0]["out"], dtype=np.float32).reshape(N)


# revision 14
# speedup vs baseline: 2.9564x; 1.1687x over previous
"""Trainium2 Bass kernel for a 1D Kernel Neural Operator (KNO) on a regular grid.

Reference computation (N=2048 nodes, C=32 channels, DEPTH=3):
    fq = gelu([f_x, x] @ lift_W.T + lift_b)
    for i in 0..2:
        skip  = fq @ pw_W[i].T + pw_b[i]
        K_c   = sig2_c * exp(-(x_n - x_q)^2 * a_c),  a_c = 1/(2*ell2_c)
        integ = einsum('cnq,qc->nc', K, fq * w)
        fq    = skip + integ; gelu if i < 2
    out = (gelu(gelu(fq@W1.T+b1)@W2.T+b2)) @ W3.T + b3

Instead of materializing the C x N x N kernels, we use the factorization
exp(-a(x_n-x_q)^2) = e^{-a x_n^2} e^{2 a x_n x_q} e^{-a x_q^2} and the Taylor
expansion e^{2a x_n x_q} = sum_k (2a)^k/k! x_n^k x_q^k (K=32 terms; the
neglected tail is < ~1e-5 relative for the a-range this problem generates).
Each layer's integral becomes two small matmuls through the moment basis:
    U      = fq ⊙ e^{-a_c x_q^2}                     [N,C]
    M[k,c] = sum_q (w_q x_q^k) U[q,c]               [K,C]   (PE matmul)
    Mt     = M ⊙ B,  B[k,c] = (2a_c)^k/k!
    integ  = (s2_c e^{-a_c x_n^2}) ⊙ (V @ Mt)       [N,C]   (PE matmul)

All x-grid/parameter-derived constants (moment basis V, its transpose, the
exponential prefactor tensors, scaled weights) are precomputed host-side in
float64 and shipped in three batched DMA blocks, so the device program is just
the f_x-dependent dataflow: ~90 engine instructions, all matmuls in bf16
(single-pass PE mode, fp32 PSUM accumulate), gelu the only activation function.

Data layout: the [N,C] state lives channel-transposed in a 4-chunk stack
fqT4[c + 32j, n'] = fq[512j + n', c], a single [128, 512] SBUF tile. Channel
mixing / eval / final projection each run as 4 concurrent quadrant matmuls.

Sharding: the whole problem is a ~30us chain of dependent small ops, so all 8
cores run identical replicas (collectives would cost more than they save); the
output is taken from core 0.
"""

import numpy as np

import concourse.bass as bass
import concourse.tile as tile
from concourse import bacc, mybir
from concourse.bass_utils import run_bass_kernel_spmd

N = 2048
C = 32
K = 32
DEPTH = 3
NCORES = 8
NT = N // 128             # 16 q-tiles of 128
NCHUNK = N // 512         # 4 chunks of 512
F32 = mybir.dt.float32
BF16 = mybir.dt.bfloat16
NPBF16 = mybir.dt.np(mybir.dt.bfloat16)
AF = mybir.ActivationFunctionType
ALU = mybir.AluOpType

# pack1 column offsets
P_IDENT = 0
P_V4 = P_IDENT + 128          # V4[p, 32t+k] = w_q x_q^k, q = 128t+p
P_PWW = P_V4 + NT * K         # pwWT3[c+32j, 32i+c'] = pw_W[i][c',c]
P_P1W = P_PWW + 3 * C
P_P2W = P_P1W + C
P_B4 = P_P2W + C              # B4_3[32s+k, 32i+c] = (2a_ic)^k/k!
P_P3W = P_B4 + 3 * C
P_PWB = P_P3W + 4             # pwb3[c+32j, i] = pw_b[i][c]
P_P1B = P_PWB + 3
P_P2B = P_P1B + 1
P_P3B = P_P2B + 1
P_LB = P_P3B + 1              # liftb4[c+32j] = lift_b[c]
P_LW = P_LB + 1               # liftW[r, c] = lift_W[c, r], rows 0..1
W1 = P_LW + C
WC = 8                        # packc (f32): pwb3 | p1b | p2b | p3b | liftb
PC_PWB = 0
PC_P1B = 3
PC_P2B = 4
PC_P3B = 5
PC_LB = 6
W2 = 1024                     # pack2: tmpN_0 | Es_0
W3 = 4 * 512                  # pack3: tmpN_1 | Es_1 | tmpN_2 | Es_2

_CACHE = {}


def build_program(nc):
    pack1 = nc.dram_tensor("pack1", [128, W1], BF16, kind="ExternalInput")
    pack2 = nc.dram_tensor("pack2", [128, W2], BF16, kind="ExternalInput")
    pack3 = nc.dram_tensor("pack3", [128, W3], BF16, kind="ExternalInput")
    lift_in = nc.dram_tensor("lift_in", [2, N], BF16, kind="ExternalInput")
    vt_dram = nc.dram_tensor("vt", [K, N], BF16, kind="ExternalInput")
    packc = nc.dram_tensor("packc", [128, WC], F32, kind="ExternalInput")
    out_dram = nc.dram_tensor("out", [1, N], F32, kind="ExternalOutput")

    with tile.TileContext(nc) as tc:
        with (
            tc.tile_pool(name="const", bufs=1) as cp,
            tc.tile_pool(name="work", bufs=2) as wp,
            tc.tile_pool(name="psA", bufs=2, space="PSUM") as ppa,
            tc.tile_pool(name="psB", bufs=2, space="PSUM") as ppb,
            tc.tile_pool(name="psC", bufs=2, space="PSUM") as ppc,
            tc.tile_pool(name="psS", bufs=2, space="PSUM") as pps,
        ):
            # ------- batched input loads on four different queues -------
            pc = cp.tile([128, WC], F32, tag="pc")
            nc.sync.dma_start(pc[:], packc[:])
            liftsb = cp.tile([2, N], BF16, tag="liftsb")
            nc.gpsimd.dma_start(liftsb[:], lift_in[:])
            p1 = cp.tile([128, W1], BF16, tag="p1")
            nc.sync.dma_start(p1[:], pack1[:])
            p2 = cp.tile([128, W2], BF16, tag="p2")
            nc.gpsimd.dma_start(p2[:], pack2[:])
            vt = cp.tile([K, N], BF16, tag="vt")
            nc.gpsimd.dma_start(vt[:], vt_dram[:])
            p3 = cp.tile([128, W3], BF16, tag="p3")
            nc.sync.dma_start(p3[:], pack3[:])

            ident = p1[:, P_IDENT : P_IDENT + 128]
            V4 = p1[:, P_V4 : P_V4 + NT * K]

            def tmpN(i):
                src = p2 if i == 0 else p3
                off = 0 if i == 0 else 1024 * (i - 1)
                return src[:, off : off + 512]

            def Es(i):
                src = p2 if i == 0 else p3
                off = 512 if i == 0 else 1024 * (i - 1) + 512
                return src[:, off : off + 512]

            # ---------------- lift ----------------
            liftp = ppa.tile([128, 512], F32, tag="mix")
            for j in range(NCHUNK):
                nc.tensor.matmul(
                    liftp[32 * j : 32 * (j + 1), :],
                    p1[0:2, P_LW : P_LW + C],
                    liftsb[:, 512 * j : 512 * (j + 1)],
                    start=True,
                    stop=True,
                    tile_position=(0, 32 * j),
                )
            fq = wp.tile([128, 512], BF16, tag="fq")
            nc.scalar.activation(
                fq[:], liftp[:], AF.Gelu_apprx_tanh, bias=pc[:, PC_LB : PC_LB + 1]
            )

            # ---------------- KNO layers ----------------
            for i in range(DEPTH):
                # skip^T (bias folded into the combine/gelu)
                skp = ppa.tile([128, 512], F32, tag="mix")
                for j in range(NCHUNK):
                    nc.tensor.matmul(
                        skp[32 * j : 32 * (j + 1), :],
                        p1[32 * j : 32 * (j + 1), P_PWW + C * i : P_PWW + C * (i + 1)],
                        fq[32 * j : 32 * (j + 1), :],
                        start=True,
                        stop=True,
                        tile_position=(32 * j, 32 * j),
                    )
                # transpose back to natural layout, U = fq_nat * e^{-a x_q^2}
                trp = ppb.tile([128, 512], BF16, tag="trp")
                for m in range(4):
                    nc.tensor.transpose(
                        trp[:, 128 * m : 128 * (m + 1)],
                        fq[:, 128 * m : 128 * (m + 1)],
                        ident,
                    )
                U = wp.tile([128, 512], BF16, tag="U")
                nc.vector.tensor_mul(U[:], trp[:], tmpN(i))
                # moments M[k,c] = sum_q (w_q x_q^k) U[q,c]
                Mp = pps.tile([K, C], F32, tag="Mp")
                for t in range(NT):
                    m, j = t % 4, t // 4
                    nc.tensor.matmul(
                        Mp[:],
                        V4[:, K * t : K * (t + 1)],
                        U[:, 128 * m + 32 * j : 128 * m + 32 * j + 32],
                        start=(t == 0),
                        stop=(t == NT - 1),
                    )
                # Mt: B-scaled moments
                Mt = wp.tile([K, C], BF16, tag="Mt")
                nc.vector.tensor_mul(
                    Mt[:], Mp[:], p1[0:K, P_B4 + C * i : P_B4 + C * (i + 1)]
                )
                # eval: PT[c + 32s, n'] = sum_k Mt[k,c] x_n^k, n = 512s + n'
                PT = ppc.tile([128, 512], F32, tag="PT")
                for s in range(4):
                    nc.tensor.matmul(
                        PT[32 * s : 32 * (s + 1), :],
                        Mt[:],
                        vt[:, 512 * s : 512 * (s + 1)],
                        start=True,
                        stop=True,
                        tile_position=(0, 32 * s),
                    )
                # combine: fq_next = gelu(skip + pw_b + Es * PT)
                z = wp.tile([128, 512], F32, tag="z")
                nc.vector.tensor_mul(z[:], PT[:], Es(i))
                if i < DEPTH - 1:
                    pre = wp.tile([128, 512], F32, tag="pre")
                    nc.vector.tensor_add(pre[:], z[:], skp[:])
                    fq = wp.tile([128, 512], BF16, tag="fq")
                    nc.scalar.activation(
                        fq[:], pre[:], AF.Gelu_apprx_tanh,
                        bias=pc[:, PC_PWB + i : PC_PWB + i + 1],
                    )
                else:
                    fq = wp.tile([128, 512], BF16, tag="fq")
                    nc.vector.scalar_tensor_tensor(
                        fq[:], z[:], pc[:, PC_PWB + i : PC_PWB + i + 1], skp[:],
                        ALU.add, ALU.add,
                    )

            # ---------------- projection head ----------------
            for woff, boff in ((P_P1W, PC_P1B), (P_P2W, PC_P2B)):
                ppj = ppa.tile([128, 512], F32, tag="mix")
                for j in range(NCHUNK):
                    nc.tensor.matmul(
                        ppj[32 * j : 32 * (j + 1), :],
                        p1[32 * j : 32 * (j + 1), woff : woff + C],
                        fq[32 * j : 32 * (j + 1), :],
                        start=True,
                        stop=True,
                        tile_position=(32 * j, 32 * j),
                    )
                nxt = wp.tile([128, 512], BF16, tag="fq")
                nc.scalar.activation(
                    nxt[:], ppj[:], AF.Gelu_apprx_tanh, bias=pc[:, boff : boff + 1]
                )
                fq = nxt

            p3p = ppc.tile([4, 512], F32, tag="PT")
            nc.tensor.matmul(
                p3p[:],
                p1[:, P_P3W : P_P3W + 4],
                fq[:],
                start=True,
                stop=True,
            )
            outsb = wp.tile([4, 512], F32, tag="outsb")
            nc.vector.tensor_scalar_add(
                outsb[:], p3p[:], pc[0:4, PC_P3B : PC_P3B + 1]
            )
            nc.sync.dma_start(
                out_dram[:].rearrange("o (b c) -> (o b) c", b=4), outsb[:]
            )

    return nc


def get_nc():
    if "nc" not in _CACHE:
        nc = bacc.Bacc("TRN2", target_bir_lowering=False, debug=False, num_devices=NCORES)
        build_program(nc)
        nc.compile()
        _CACHE["nc"] = nc
    return _CACHE["nc"]


def make_in_map(
    f_x, x_grid, q_weights, lift_W, lift_b, pw_W, pw_b, ker_log_ell, ker_log_sigma,
    proj1_W, proj1_b, proj2_W, proj2_b, proj3_W, proj3_b,
):
    f4 = lambda a: np.asarray(a, dtype=np.float64)
    x = f4(x_grid).reshape(N)
    w = f4(q_weights).reshape(N)
    ks = np.arange(K, dtype=np.float64)
    lnfact = np.concatenate([[0.0], np.cumsum(np.log(np.arange(1.0, K)))])
    a = 0.5 * np.exp(-2.0 * f4(ker_log_ell))        # [DEPTH, C]
    sig2 = np.exp(2.0 * f4(ker_log_sigma))          # [DEPTH, C]

    pack1 = np.zeros((128, W1), dtype=np.float32)
    pack1[:, P_IDENT : P_IDENT + 128] = np.eye(128, dtype=np.float32)
    # V4[p, 32t+k] = w_q x_q^k, q = 128t + p
    V = w[:, None] * np.power(x[:, None], ks[None, :])          # [N, K]
    pack1[:, P_V4 : P_V4 + NT * K] = (
        V.reshape(NT, 128, K).transpose(1, 0, 2).reshape(128, NT * K)
    ).astype(np.float32)
    # vt[k, n] = x_n^k
    VT = np.power(x[None, :], ks[:, None]).astype(np.float32)   # [K, N]
    # pwWT3[c+32j, 32i+c'] = pw_W[i][c', c]
    pwT = f4(pw_W).transpose(0, 2, 1)                           # [D, C, C']
    pack1[:, P_PWW : P_PWW + 3 * C] = np.tile(
        pwT.transpose(1, 0, 2).reshape(C, 3 * C), (4, 1)
    ).astype(np.float32)
    pack1[:, P_P1W : P_P1W + C] = np.tile(f4(proj1_W).T, (4, 1)).astype(np.float32)
    pack1[:, P_P2W : P_P2W + C] = np.tile(f4(proj2_W).T, (4, 1)).astype(np.float32)
    # B4_3[32s+k, 32i+c] = (2 a_ic)^k / k!
    B = np.exp(ks[None, :, None] * np.log(2.0 * a)[:, None, :] - lnfact[None, :, None])
    pack1[:, P_B4 : P_B4 + 3 * C] = np.tile(
        B.transpose(1, 0, 2).reshape(K, 3 * C), (4, 1)
    ).astype(np.float32)
    p3w4 = np.zeros((128, 4), dtype=np.float32)
    for j in range(4):
        p3w4[32 * j : 32 * (j + 1), j] = f4(proj3_W)[0]
    pack1[:, P_P3W : P_P3W + 4] = p3w4
    pack1[0:2, P_LW : P_LW + C] = f4(lift_W).T.astype(np.float32)
    packc = np.zeros((128, WC), dtype=np.float32)
    packc[:, PC_PWB : PC_PWB + 3] = np.tile(f4(pw_b).T, (4, 1)).astype(np.float32)
    packc[:, PC_P1B] = np.tile(f4(proj1_b), 4).astype(np.float32)
    packc[:, PC_P2B] = np.tile(f4(proj2_b), 4).astype(np.float32)
    packc[:, PC_P3B] = np.float32(f4(proj3_b)[0])
    packc[:, PC_LB] = np.tile(f4(lift_b), 4).astype(np.float32)

    # exponential prefactors per layer
    # tmpN[p, 128m+32j+c] = exp(-a_c x_q^2), q = 512j+128m+p
    # Es[c+32j, n'] = sig2_c exp(-a_c x_n^2), n = 512j+n'
    x2 = x * x
    packs = []
    for i in range(DEPTH):
        eN = np.exp(-x2[:, None] * a[i][None, :])               # [N, C]
        tm = (
            eN.reshape(NCHUNK, 4, 128, C).transpose(2, 1, 0, 3).reshape(128, 512)
        ).astype(np.float32)
        es = (
            (sig2[i][None, :] * eN).reshape(NCHUNK, 512, C)
            .transpose(0, 2, 1).reshape(128, 512)
        ).astype(np.float32)
        packs.append((tm, es))
    pack2 = np.concatenate(packs[0], axis=1)
    pack3 = np.concatenate([t for pr in packs[1:] for t in pr], axis=1)

    lift_arr = np.stack(
        [np.asarray(f_x, np.float32).reshape(N), x.astype(np.float32)]
    )
    return {
        "pack1": pack1.astype(NPBF16),
        "pack2": np.ascontiguousarray(pack2).astype(NPBF16),
        "pack3": np.ascontiguousarray(pack3).astype(NPBF16),
        "lift_in": np.ascontiguousarray(lift_arr).astype(NPBF16),
        "vt": np.ascontiguousarray(VT).astype(NPBF16),
        "packc": packc,
    }


def kernel(**inputs) -> np.ndarray:
    nc = get_nc()
    in_map = make_in_map(**inputs)
    res = run_bass_kernel_spmd(nc, [in_map] * NCORES, list(range(NCORES)))
    return np.asarray(res.results[0]["out"], dtype=np.float32).reshape(N)
